# revision 1
# baseline (speedup 1.0000x reference)
"""GCN layer with virtual node on 8 Trainium2 NeuronCores (Bass/Tile).

Reference computation (fp32):
    agg = segment_sum(H[src], dst, N)        # message passing
    out = H + agg
    vmean = segment_mean(out, batch, G)      # virtual node
    out = out + vmean[batch]
    y = relu(out @ W)

Distribution strategy (self-contained, hardcoded):
  - batch is sorted, G=256 graphs, 8 cores -> core c owns graphs
    [32c, 32c+32) == a contiguous node range (graph-aligned node sharding).
    Per-graph means never cross cores: no collectives needed.
  - Edges are partitioned by owning core of dst (host-side graph partitioning,
    index arithmetic only). Within a core: 128-dst windows; per window the
    edges' source rows are fetched with dma_gather (int16 indices -> 4 source
    "classes" of <=32768 rows each; per-(window,class) call with a
    register-driven valid count so padding costs no DMA traffic).
  - segment_sum via PE one-hot matmul: R_t[s, m] = (dst_rel[s] == m) built on
    DVE (is_equal vs iota); psum_w[dst128, feat128] += R_t^T @ G_t.
  - virtual node: B_w[s, g] = (batch_rel[s] == g); psum_s[g, feat] += B_w^T @
    out_w accumulated over windows; vmean = psum_s * (1/count); broadcast back
    with vb_w = (B_w^T)^T ... = matmul(lhsT=transpose(B_w), rhs=vmean).
  - final: y_w = relu((out2_w^T)^T @ W) via PE transpose + matmul.
"""
import os
import numpy as np

from concourse import bass, bacc, mybir
import concourse.tile as tile
from concourse.bass_utils import run_bass_kernel_spmd

P = 128
N_CORES = 8
D = 128
F32 = mybir.dt.float32
I32 = mybir.dt.int32
I16 = mybir.dt.int16
F16 = mybir.dt.float16
CLASS_SIZE = 32768  # int16 index reach for dma_gather


def _ceil(a, b):
    return -(-a // b)


# ---------------------------------------------------------------------------
# host-side prep: pure index arithmetic / sharding metadata
# ---------------------------------------------------------------------------

def _prep(H, edge_index, batch, n_graphs):
    N = H.shape[0]
    src = np.asarray(edge_index[0], dtype=np.int64)
    dst = np.asarray(edge_index[1], dtype=np.int64)
    batch = np.asarray(batch, dtype=np.int64)
    gpc = n_graphs // N_CORES  # graphs per core
    n_cls = _ceil(N, CLASS_SIZE)

    gstart = np.searchsorted(batch, np.arange(n_graphs + 1))  # node start per graph
    core_start = gstart[::gpc]  # [N_CORES+1]
    counts = np.diff(gstart)

    node_core = (batch // gpc).astype(np.int64)
    ecore = node_core[dst]

    n_c = np.diff(core_start)  # nodes per core
    NW = int(_ceil(n_c.max(), P))  # windows per core (shared)

    # per-core, per-window, per-class edge buckets
    percore = []
    max_cnt = np.zeros(n_cls, dtype=np.int64)  # max ceil16 count per class
    for c in range(N_CORES):
        m = ecore == c
        s_c, d_c = src[m], dst[m]
        dstl = d_c - core_start[c]
        w = dstl >> 7
        k = s_c // CLASS_SIZE
        order = np.lexsort((s_c, k, w))
        s_c, dstl, w, k = s_c[order], dstl[order], w[order], k[order]
        # counts per (w, k)
        wk = w * n_cls + k
        cnt = np.bincount(wk, minlength=NW * n_cls).reshape(NW, n_cls)
        cntv = np.maximum(cnt, 1)  # >=1 valid entry per run (idx-0 pad)
        max_cnt = np.maximum(max_cnt, cntv.max(axis=0))
        percore.append((s_c, dstl, w, k, cnt, cntv))

    cap = (np.ceil(max_cnt / P).astype(np.int64) * P)  # slots per class (128-mult)
    cap_tiles = cap // P
    KW = int(cap_tiles.sum())  # tiles per window
    t0 = np.concatenate([[0], np.cumsum(cap_tiles)])  # tile offset per class

    params = dict(
        N=N, NW=NW, KW=KW, gpc=gpc, n_cls=n_cls,
        cap=tuple(int(x) for x in cap),
        cls_size=tuple(min(CLASS_SIZE, N - CLASS_SIZE * k) for k in range(n_cls)),
    )

    in_maps = []
    T = NW * KW
    for c in range(N_CORES):
        s_c, dstl, w, k, cnt, cntv = percore[c]
        idx16 = np.full((T * P // 16 * 1,), -1, dtype=np.int64)  # flat slot idx
        idx_flat = np.full(T * P, -1, dtype=np.int64)
        drel = np.full((T * P,), -1.0, dtype=np.float32)
        cnts = cntv.astype(np.int32)  # [NW, n_cls] register values
        # fill slots
        pos = 0
        ptr = np.zeros((NW, n_cls), dtype=np.int64)
        # edges are sorted by (w, k); compute slot base per (w, k) run
        starts = np.concatenate([[0], np.cumsum(cnt.ravel())]).astype(np.int64)
        for wi in range(NW):
            wbase = wi * KW * P
            for ki in range(params["n_cls"]):
                run0 = starts[wi * n_cls + ki]
                n = cnt[wi, ki]
                sbase = wbase + int(t0[ki]) * P
                if n:
                    sl = np.arange(n) + sbase
                    idx_flat[sl] = s_c[run0:run0 + n] - CLASS_SIZE * ki
                    drel[sl] = (dstl[run0:run0 + n] - (wi << 7)).astype(np.float32)
                # idx-0 pads counted as valid up to cntv
                npad = cntv[wi, ki] - n
                if npad:
                    sl = np.arange(n, n + npad) + sbase
                    idx_flat[sl] = 0  # gather class base row; drel stays -1
        # wrap idx per (w, k) call region into [16, cap/16] blocks
        wrapped = np.full((P, T * P // 16), -1, dtype=np.int16)
        for ki in range(n_cls):
            ccap = params["cap"][ki]
            colbase0 = int(t0[ki]) * P // 16
            for wi in range(NW):
                sbase = wi * KW * P + int(t0[ki]) * P
                block = idx_flat[sbase:sbase + ccap]
                wb = block.reshape(ccap // 16, 16).T.astype(np.int16)
                col0 = wi * (KW * P // 16) + colbase0
                wrapped[:16, col0:col0 + ccap // 16] = wb
        wrapped[16:] = np.tile(wrapped[:16], (7, 1))

        drel2 = drel.reshape(T, P).T.astype(np.float16)  # [128, T] slot (p, t)

        nodes = int(n_c[c])
        hcore = np.zeros((NW * P, D), dtype=np.float32)
        hcore[:nodes] = H[core_start[c]:core_start[c] + nodes]
        brel = np.full((NW * P,), -1.0, dtype=np.float32)
        brel[:nodes] = (batch[core_start[c]:core_start[c] + nodes]
                        - c * gpc).astype(np.float32)
        brel2 = brel.reshape(NW, P).T.copy()  # [128, NW]
        invc = (1.0 / np.maximum(counts[c * gpc:(c + 1) * gpc], 1)).astype(
            np.float32)[:, None]

        in_maps.append({
            "hfull": np.ascontiguousarray(H, dtype=np.float32),
            "hcore": hcore,
            "idx16": np.ascontiguousarray(wrapped),
            "cnt": np.ascontiguousarray(cnts.reshape(1, NW * n_cls)),
            "drel": np.ascontiguousarray(drel2),
            "brel": np.ascontiguousarray(brel2),
            "invc": invc,
        })
    return params, in_maps, n_c, core_start


def _consts(params, W):
    iota128 = np.broadcast_to(np.arange(P, dtype=np.float16), (P, P)).copy()
    iotag = np.broadcast_to(np.arange(params["gpc"], dtype=np.float32),
                            (P, params["gpc"])).copy()
    ident = np.eye(P, dtype=np.float32)
    return {"iota128": iota128, "iotag": iotag, "ident": ident,
            "wmat": np.ascontiguousarray(W, dtype=np.float32)}


# ---------------------------------------------------------------------------
# device kernel builder (SPMD: one program, per-core data)
# ---------------------------------------------------------------------------

def _build(params, stage=4):
    NW, KW = params["NW"], params["KW"]
    gpc, n_cls = params["gpc"], params["n_cls"]
    cap = params["cap"]
    cls_size = params["cls_size"]
    cap_tiles = [c // P for c in cap]
    t0 = np.concatenate([[0], np.cumsum(cap_tiles)]).astype(int)
    T = NW * KW
    N = params["N"]

    nc = bacc.Bacc("TRN2", target_bir_lowering=False, debug=False,
                   num_devices=N_CORES)
    hfull_d = nc.dram_tensor("hfull", [N, D], F32, kind="ExternalInput")
    hcore_d = nc.dram_tensor("hcore", [NW * P, D], F32, kind="ExternalInput")
    idx_d = nc.dram_tensor("idx16", [P, T * P // 16], I16, kind="ExternalInput")
    cnt_d = nc.dram_tensor("cnt", [1, NW * n_cls], I32, kind="ExternalInput")
    drel_d = nc.dram_tensor("drel", [P, T], F16, kind="ExternalInput")
    brel_d = nc.dram_tensor("brel", [P, NW], F32, kind="ExternalInput")
    invc_d = nc.dram_tensor("invc", [gpc, 1], F32, kind="ExternalInput")
    iota128_d = nc.dram_tensor("iota128", [P, P], F16, kind="ExternalInput")
    iotag_d = nc.dram_tensor("iotag", [P, gpc], F32, kind="ExternalInput")
    ident_d = nc.dram_tensor("ident", [P, P], F32, kind="ExternalInput")
    w_d = nc.dram_tensor("wmat", [P, D], F32, kind="ExternalInput")
    y_d = nc.dram_tensor("y", [NW * P, D], F32, kind="ExternalOutput")

    IDXC = KW * P // 16  # idx cols per window

    with tile.TileContext(nc) as tc:
        with tc.tile_pool(name="const", bufs=1) as cpool:
            iota128_t = cpool.tile([P, P], F16)
            nc.sync.dma_start(out=iota128_t[:], in_=iota128_d[:])
            iotag_t = cpool.tile([P, gpc], F32)
            nc.sync.dma_start(out=iotag_t[:], in_=iotag_d[:])
            ident_t = cpool.tile([P, P], F32)
            nc.sync.dma_start(out=ident_t[:], in_=ident_d[:])
            w_t = cpool.tile([P, D], F32)
            nc.sync.dma_start(out=w_t[:], in_=w_d[:])
            invc_t = cpool.tile([gpc, 1], F32)
            nc.sync.dma_start(out=invc_t[:], in_=invc_d[:])
            drel_t = cpool.tile([P, T], F16)
            nc.sync.dma_start(out=drel_t[:], in_=drel_d[:])
            brel_t = cpool.tile([P, NW], F32)
            nc.sync.dma_start(out=brel_t[:], in_=brel_d[:])
            cnt_t = cpool.tile([1, NW * n_cls], I32)
            nc.sync.dma_start(out=cnt_t[:], in_=cnt_d[:])
            idx_t = cpool.tile([P, T * P // 16], I16)
            nc.sync.dma_start(out=idx_t[:], in_=idx_d[:])

            out_sb = cpool.tile([P, NW, D], F32)
            b_all = cpool.tile([P, NW, gpc], F32)
            vmean_t = cpool.tile([gpc, D], F32)

            with tc.tile_pool(name="gpool", bufs=3) as gpool, \
                 tc.tile_pool(name="g16pool", bufs=3) as g16pool, \
                 tc.tile_pool(name="hpool", bufs=3) as hpool, \
                 tc.tile_pool(name="rpool", bufs=4) as rpool, \
                 tc.tile_pool(name="pw", bufs=4, space="PSUM") as pwpool, \
                 tc.tile_pool(name="ps", bufs=1, space="PSUM") as pspool, \
                 nc.gpsimd.register("gcnt") as gcnt:

                psum_s = pspool.tile([gpc, D], F32, space="PSUM")

                for w in range(NW):
                    g_t = gpool.tile([P, KW, D], F32, tag="G")
                    if w < 3:
                        # pad slots must be finite: 0 * NaN would poison the
                        # one-hot matmul. After the first rotation of the 3
                        # G buffers, stale content is old gathered rows
                        # (finite), so no re-zeroing is needed.
                        nc.vector.memset(g_t[:], 0.0)
                    for k in range(n_cls):
                        nc.gpsimd.load(
                            gcnt, cnt_t[0:1, w * n_cls + k:w * n_cls + k + 1])
                        base = CLASS_SIZE * k
                        nc.gpsimd.dma_gather(
                            out_ap=g_t[:, int(t0[k]):int(t0[k + 1]), :],
                            in_ap=hfull_d[base:base + cls_size[k], :],
                            idxs_ap=idx_t[:, w * IDXC + int(t0[k]) * 8:
                                          w * IDXC + int(t0[k + 1]) * 8],
                            num_idxs=cap[k],
                            num_idxs_reg=gcnt,
                            elem_size=D,
                            single_packet=False,
                        )
                    if stage == 0:
                        nc.vector.tensor_copy(out_sb[:, w, :], g_t[:, 0, :])
                        continue
                    g16_t = g16pool.tile([P, KW, D], F16, tag="G16")
                    nc.vector.tensor_copy(g16_t[:], g_t[:])
                    psum_w = pwpool.tile([P, D], F32, space="PSUM", tag="pw")
                    for t in range(KW):
                        r_t = rpool.tile([P, P], F16, tag="R")
                        nc.vector.tensor_tensor(
                            out=r_t[:],
                            in0=drel_t[:, w * KW + t:w * KW + t + 1
                                       ].to_broadcast([P, P]),
                            in1=iota128_t[:],
                            op=mybir.AluOpType.is_equal)
                        nc.tensor.matmul(
                            psum_w[:], r_t[:], g16_t[:, t, :],
                            start=(t == 0), stop=(t == KW - 1))
                    if stage == 1:
                        nc.vector.tensor_copy(out_sb[:, w, :], psum_w[:])
                        continue
                    hc_t = hpool.tile([P, D], F32, tag="H")
                    nc.sync.dma_start(out=hc_t[:],
                                      in_=hcore_d[w * P:(w + 1) * P, :])
                    nc.vector.tensor_tensor(
                        out=out_sb[:, w, :], in0=psum_w[:], in1=hc_t[:],
                        op=mybir.AluOpType.add)
                    if stage >= 3:
                        nc.vector.tensor_tensor(
                            out=b_all[:, w, :],
                            in0=brel_t[:, w:w + 1].to_broadcast([P, gpc]),
                            in1=iotag_t[:],
                            op=mybir.AluOpType.is_equal)
                        nc.tensor.matmul(
                            psum_s[:], b_all[:, w, :], out_sb[:, w, :],
                            start=(w == 0), stop=(w == NW - 1),
                            skip_group_check=True)

                if stage >= 3:
                    nc.vector.tensor_scalar(
                        vmean_t[:], psum_s[:], invc_t[:], None,
                        mybir.AluOpType.mult)

            if stage < 4:
                for w in range(NW):
                    nc.sync.dma_start(out=y_d[w * P:(w + 1) * P, :],
                                      in_=out_sb[:, w, :])
            with tc.tile_pool(name="p3", bufs=4) as p3, \
                 tc.tile_pool(name="pp3", bufs=2, space="PSUM") as pp3:
                for w in range(NW if stage >= 4 else 0):
                    psum_bt = pp3.tile([gpc, P], F32, space="PSUM", tag="pbt")
                    nc.tensor.transpose(psum_bt[:], b_all[:, w, :], ident_t[:])
                    b2_t = p3.tile([gpc, P], F32, tag="B2")
                    nc.vector.tensor_copy(b2_t[:], psum_bt[:])
                    psum_vb = pp3.tile([P, D], F32, space="PSUM", tag="pvb")
                    nc.tensor.matmul(psum_vb[:], b2_t[:], vmean_t[:],
                                     start=True, stop=True)
                    o2_t = p3.tile([P, D], F32, tag="O2")
                    nc.vector.tensor_tensor(
                        out=o2_t[:], in0=out_sb[:, w, :], in1=psum_vb[:],
                        op=mybir.AluOpType.add)
                    psum_t = pp3.tile([P, P], F32, space="PSUM", tag="pt")
                    nc.tensor.transpose(psum_t[:], o2_t[:], ident_t[:])
                    t_t = p3.tile([P, P], F32, tag="T")
                    nc.vector.tensor_copy(t_t[:], psum_t[:])
                    psum_y = pp3.tile([P, D], F32, space="PSUM", tag="py")
                    nc.tensor.matmul(psum_y[:], t_t[:], w_t[:],
                                     start=True, stop=True)
                    y_t = p3.tile([P, D], F32, tag="Y")
                    nc.scalar.activation(y_t[:], psum_y[:],
                                         mybir.ActivationFunctionType.Relu)
                    nc.sync.dma_start(out=y_d[w * P:(w + 1) * P, :],
                                      in_=y_t[:])
    _finish_compile(nc)
    return nc


def _finish_compile(nc):
    nc.compile()
    # compile()'s tail passes (library-load insertion for the custom DMA
    # instructions) can reintroduce >1 sync wait per instruction, which the
    # TRN2 ISA rejects. Re-split and re-codegen.
    import bass_rust
    bass_rust.generate_event_semaphores(nc)
    nc.codegen_inst_isa_subclasses()


_BUILD_CACHE = {}


def _build_cached(params):
    key = tuple(sorted((k, v) for k, v in params.items()))
    if key not in _BUILD_CACHE:
        _BUILD_CACHE[key] = _build(params)
    return _BUILD_CACHE[key]


def _run(H, edge_index, batch, W, n_graphs, trace=False):
    H = np.asarray(H)
    params, in_maps, n_c, core_start = _prep(H, edge_index, batch, n_graphs)
    consts = _consts(params, np.asarray(W))
    for m in in_maps:
        m.update(consts)
    nc = _build_cached(params)
    res = run_bass_kernel_spmd(nc, in_maps, list(range(N_CORES)), trace=trace)
    N = H.shape[0]
    y = np.empty((N, D), dtype=np.float32)
    for c in range(N_CORES):
        y[core_start[c]:core_start[c] + n_c[c]] = \
            res.results[c]["y"][:n_c[c]]
    return y, res


def kernel(H, edge_index, batch, W):
    y, _ = _run(H, edge_index, batch, W, n_graphs=256,
                trace=bool(os.environ.get("GCN_TRACE")))
    return y

